# revision 25
# baseline (speedup 1.0000x reference)
"""Trainium2 Bass kernel for the ConvNet problem (v2).

Per 2048-sample super-tile: the signal is loaded feature-major straight from
HBM via xbar DMA transposes (f32 moved exactly as lo/hi uint16 planes), x^2 is
formed on the GpSimd engine, then eight 256-sample sub-tiles flow through
  windowed sums (banded matmuls, float32r) -> std -> conv1 -> conv2 -> fc1 ->
  fc2 -> pamap -> (end-batched) log_softmax.
All layer biases are folded into the matmuls via a constant-ones row that is
seeded in the std tile and propagated by unit columns, so every PSUM
evacuation is a single wide relu/copy op.  PSUM is hand-mapped onto the 8
banks with double buffers for s/s2/h2 so consecutive sub-tiles pipeline.

Sharding: pure data parallelism, batch split 8 ways across NeuronCores.
"""

import math
import os

import numpy as np

# ---------------------------------------------------------------------------
# Problem constants (hardcoded; kernel.py must be self-contained)
# ---------------------------------------------------------------------------
B_TOTAL, L, C = 131072, 50, 3
WIN = 10
NCORES = 8
B_CORE = B_TOTAL // NCORES          # 16384
G_SUPER = 16                        # samples per partition per super-tile
SUPER = 128 * G_SUPER               # 2048 samples per super-tile
N_SUPER_FULL = B_CORE // SUPER      # 8
NSUB = 256                          # samples per sub-tile (matmul N)
SUBQ = SUPER // NSUB                # 8 sub-tiles per super-tile
F_IN = L * C                        # 150
F_USE = 147                         # features actually consumed (x[49,:] unused)
F_PAD = 256                         # feature slots after padding (2x128)
CIN = F_PAD                         # bf16 columns of the prepped signal
F_STD = 120                         # 40 windows x 3 channels
F_C1 = 190                         # 38 x 5
F_C2 = 360                          # 36 x 10
F_FC1 = 256
F_FC2 = 64
F_OUT = 12

# debug knob (harness never sets this; default = full problem)
_N_SUPER = int(os.environ.get("ATRN_NSUPER", str(N_SUPER_FULL)))


# ---------------------------------------------------------------------------
# Tile drain patch: walrus in this container rejects >2 sem waits on a
# CTRL-class (Drain) instruction.  Spread the end-of-kernel global-clock waits
# across per-proc SP nops (one sem each) before an unadorned drain.
# ---------------------------------------------------------------------------
def _install_drain_patch():
    import concourse.tile as tile
    from concourse.tile_scheduler import N_PROCS
    from concourse.vector_clock import ScopedClock, VectorClock

    if getattr(tile.TileContext, "_drain_patch_installed", False):
        return

    def _patched_drain_and_barrier(self, tick_clock, wait_clock):
        nc = self.nc
        gc = tick_clock.global_clock
        for p in range(N_PROCS):
            if gc[p] <= 0:
                continue
            v = [0] * N_PROCS
            v[p] = gc[p]
            nop = nc.sync.nop()
            wait_clock.add_sem_waits(nop.ins, ScopedClock({None: VectorClock(v)}))
        nc.sync.drain()
        nc.all_engine_barrier()
        assert self.sems is not None
        popped = nc._tile_sem_poison_stack.pop()
        assert popped is self._sem_poison
        nc.clear_and_free_semaphores(list(self.sems.allocated().values()))
        nc.all_engine_barrier()

    tile.TileContext._drain_and_barrier = _patched_drain_and_barrier
    tile.TileContext._drain_patch_installed = True


def _wait_cap(ins):
    # This walrus build rejects >1 sem wait on engine instructions.
    return 1


def _split_excess_waits(nc):
    """Hoist excess sem waits onto same-engine nops inserted just before."""
    from concourse import mybir

    ctr = 0
    for f in nc.m.functions:
        for blk in f.blocks:
            il = blk.instructions
            i = 0
            while i < len(il):
                ins = il[i]
                si = ins.sync_info
                cap = _wait_cap(ins)
                if si is not None and len(si.on_wait) > cap:
                    waits = list(si.on_wait)
                    extra, keep = waits[:-cap], waits[-cap:]
                    for w in extra:
                        ctr += 1
                        nop = mybir.InstNoOp(name=f"waitsplit-{ctr}",
                                             ins=[], outs=[])
                        nop.engine = ins.engine
                        nop.sync_info = type(si)(on_wait=[w], on_update=[])
                        nc.register_instruction(nop, overwrite=True)
                        il.insert(i, nop)
                        i += 1
                    ins.sync_info = type(si)(on_wait=keep,
                                             on_update=list(si.on_update))
                i += 1


# ---------------------------------------------------------------------------
# Host-side signal prep: [B,150] f32 -> [B, 256] bf16 (zero-padded).  The
# 2-byte xbar DMA transpose then lands feature-major bf16 tiles in SBUF.
# bf16 input + f32 PSUM accumulation keeps the end-to-end max rel err at
# ~3e-3, well inside the 2e-2 gate (measured on the full dataset).
# ---------------------------------------------------------------------------
def _prep_sig(sig_flat):
    import ml_dtypes
    b = sig_flat.shape[0]
    padded = np.zeros((b, F_PAD), np.float32)
    padded[:, :F_IN] = sig_flat
    return padded.astype(ml_dtypes.bfloat16)


def _tf32(a):
    # round-to-nearest fp32 -> tf32 (10-bit mantissa), bits kept in fp32
    u = np.ascontiguousarray(a, np.float32).view(np.uint32)
    return ((u + 0x00001000) & 0xFFFFE000).astype(np.uint32).view(np.float32)


# ---------------------------------------------------------------------------
# Host-side weight preprocessing -> one [128, WF] f32 blob
# ---------------------------------------------------------------------------
class _BlobLayout:
    def __init__(self):
        self.cols = 0
        self.slots = {}

    def add(self, name, rows, cols):
        self.slots[name] = (self.cols, rows, cols)
        self.cols += cols
        return self.slots[name]


_LAY = _BlobLayout()
_LAY.add("c1_a", 121, 128)      # conv1 out 0..127; row 120 = bias
_LAY.add("c1_b", 121, 128)      # conv1 out 128..189 + ones col 62
for m in range(3):
    _LAY.add(f"c2a{m}", 128, 128)   # conv2, K = h1 feats 0..127
    _LAY.add(f"c2b{m}", 63, 128)    # conv2, K = h1 feats 128..189 + bias row
for k, kc in enumerate((128, 128, 105)):
    for m in range(2):
        _LAY.add(f"f1_{k}_{m}", kc, 128)
for k in range(2):
    _LAY.add(f"f2_{k}", 128, F_FC2)
_LAY.add("wp", F_FC2, F_OUT)    # pamap rhs (weight-streaming)
_LAY.add("b4", F_FC2, 1)
_LAY.add("bp", 128, F_OUT)      # pamap bias replicated across partitions
WF = _LAY.cols

# bf16 blob: unscaled 0/1 windowed-sum matrices (exact in bf16); the 1/10
# scaling of s'^2 is applied later via the ACT Square scale argument.
_LAY16 = _BlobLayout()
_LAY16.add("sA_a", 128, F_STD)
_LAY16.add("sA_b", 64, F_STD)
_LAY16.add("s2_a", 128, F_STD)
_LAY16.add("s2_b", 64, F_STD)
WF16 = _LAY16.cols


def _build_blob(conv1_w, conv1_b, conv2_w, conv2_b, fc1_w, fc1_b, fc2_w, fc2_b,
                pamap_w, pamap_b):
    import ml_dtypes
    blob = np.zeros((128, WF), np.float32)
    blob16 = np.zeros((128, WF16), ml_dtypes.bfloat16)

    def put(name, arr):
        off, rows, cols = _LAY.slots[name]
        assert arr.shape == (rows, cols), (name, arr.shape, (rows, cols))
        blob[:rows, off:off + cols] = arr

    def put16(name, arr):
        off, rows, cols = _LAY16.slots[name]
        assert arr.shape == (rows, cols), (name, arr.shape, (rows, cols))
        blob16[:rows, off:off + cols] = arr

    # windowed sums (unscaled 0/1 taps -> exact in bf16)
    A = np.zeros((F_USE, F_STD), np.float32)
    for m in range(F_STD):
        l, c = divmod(m, 3)
        for k in range(WIN):
            A[3 * (l + k) + c, m] = 1.0
    sb = np.zeros((64, F_STD), np.float32)
    sb[:F_USE - 128] = A[128:]
    put16("sA_a", A[:128])
    put16("sA_b", sb)
    put16("s2_a", A[:128])
    put16("s2_b", sb)

    # conv1 as dense [in 120, out 190]; bias in row 120 (fed by the std ones
    # row); unit col 62 of the b-chunk regenerates the ones row in h1.
    M1 = np.zeros((F_STD, F_C1), np.float32)
    b1 = np.zeros(F_C1, np.float32)
    for t in range(38):
        for o in range(5):
            b1[5 * t + o] = conv1_b[o]
            for k in range(3):
                for i in range(3):
                    M1[3 * (t + k) + i, 5 * t + o] = conv1_w[o, i, k]
    c1a = np.zeros((121, 128), np.float32)
    c1a[:F_STD] = M1[:, :128]
    c1a[120] = b1[:128]
    put("c1_a", c1a)
    c1b = np.zeros((121, 128), np.float32)
    c1b[:F_STD, :F_C1 - 128] = M1[:, 128:]
    c1b[120, :F_C1 - 128] = b1[128:]
    c1b[120, 62] = 1.0
    put("c1_b", c1b)

    # conv2 as dense [in 190, out 360]; K rows 0..127 from h1a cols, rows
    # 128..189 + bias from the h1b chunk (ones at h1 row 62 of that chunk).
    M2 = np.zeros((F_C1, F_C2), np.float32)
    b2 = np.zeros(F_C2, np.float32)
    for t in range(36):
        for o in range(10):
            b2[10 * t + o] = conv2_b[o]
            for k in range(3):
                for i in range(5):
                    M2[5 * (t + k) + i, 10 * t + o] = conv2_w[o, i, k]
    mo_off = (0, 128, 256, 360)
    for m in range(3):
        mo, hi = mo_off[m], mo_off[m + 1]
        ca = np.zeros((128, 128), np.float32)
        ca[:, :hi - mo] = M2[:128, mo:hi]
        put(f"c2a{m}", ca)
        cb = np.zeros((63, 128), np.float32)
        cb[:62, :hi - mo] = M2[128:, mo:hi]
        cb[62, :hi - mo] = b2[mo:hi]
        if m == 2:
            cb[62, 104] = 1.0      # ones row for the fc1 bias fold
        put(f"c2b{m}", cb)

    # fc1 [360 -> 256]; bias in row 104 of the third K chunk (h2 ones row).
    F1 = fc1_w.T.astype(np.float32)          # [360, 256]
    for m in range(2):
        put(f"f1_0_{m}", F1[0:128, m * 128:(m + 1) * 128])
        put(f"f1_1_{m}", F1[128:256, m * 128:(m + 1) * 128])
        f12 = np.zeros((105, 128), np.float32)
        f12[:104] = F1[256:, m * 128:(m + 1) * 128]
        f12[104] = fc1_b[m * 128:(m + 1) * 128]
        put(f"f1_2_{m}", f12)

    F2 = fc2_w.T.astype(np.float32)          # [256, 64]
    for k in range(2):
        put(f"f2_{k}", F2[k * 128:(k + 1) * 128])
    put("wp", pamap_w.T.astype(np.float32))  # [64, 12]
    put("b4", fc2_b[:, None].astype(np.float32))
    put("bp", np.tile(pamap_b.astype(np.float32)[None, :], (128, 1)))
    # pre-round the matmul weights to tf32 so the on-chip FP32r rounding
    # pass is an exact no-op and CoreSim matches hardware numerics.
    blob = _tf32(blob)
    return blob, blob16


# ---------------------------------------------------------------------------
# Bass program
# ---------------------------------------------------------------------------
_PROGRAM = None


def _w(weights, name):
    off, rows, cols = _LAY.slots[name]
    return weights[0:rows, off:off + cols]


_w_raw = _w


def _w16(weights16, name):
    off, rows, cols = _LAY16.slots[name]
    return weights16[0:rows, off:off + cols]


# PSUM column map (f32 units; bank = 512 cols).  s/s2/h2 double-buffered by
# sub-tile parity; h1/h3/h4 self-cycle; lg lives for a whole super-tile.
_PS_S = (0, 256)
_PS_S2 = (512, 768)
_PS_H1 = 1024            # [128, 512]: chunk a @ +0, chunk b @ +256
_PS_H2 = (1536, 2304)    # [128, 768] each
_PS_H3 = 3072            # [128, 512]
_PS_H4 = 3584            # [64, 256]
_PS_LG = 3840            # [128, 192]


def _build_program(n_super):
    import concourse.bass as bass
    import concourse.tile as tile
    from concourse import mybir

    _install_drain_patch()
    f32 = mybir.dt.float32
    f32r = mybir.dt.float32r
    u16 = mybir.dt.uint16
    AF = mybir.ActivationFunctionType
    ALU = mybir.AluOpType

    b_core = n_super * SUPER
    nc = bass.Bass("TRN2", target_bir_lowering=False, debug=False,
                   num_devices=NCORES)
    bf16 = mybir.dt.bfloat16
    sig = nc.dram_tensor("sig", [b_core, CIN], bf16, kind="ExternalInput")
    wb = nc.dram_tensor("wb", [128, WF], f32, kind="ExternalInput")
    wb16 = nc.dram_tensor("wb16", [128, WF16], bf16, kind="ExternalInput")
    out = nc.dram_tensor("out", [b_core, F_OUT], f32, kind="ExternalOutput")

    with tile.TileContext(nc) as tc:
        import contextlib
        with contextlib.ExitStack() as ctx:
            singles = ctx.enter_context(tc.tile_pool(name="singles", bufs=1))
            sbx = ctx.enter_context(tc.tile_pool(name="sbx", bufs=2))
            sbh = ctx.enter_context(tc.tile_pool(name="sbh", bufs=3))
            pspool = ctx.enter_context(
                tc.tile_pool(name="ps", bufs=1, space="PSUM"))
            outp = ctx.enter_context(tc.tile_pool(name="outp", bufs=2))

            # DMA-loaded weights must pass through a compute op with FP32r
            # output before feeding FP32r matmuls (walrus BIR verifier rule);
            # the blob is pre-rounded to tf32 so this copy is numerically a
            # no-op.  b4/bp (non-matmul operands) are sliced from the raw
            # f32 tile.
            weights16 = singles.tile([128, WF16], bf16)
            nc.sync.dma_start(out=weights16, in_=wb16[:, :])
            weights_raw = singles.tile([128, WF], f32)
            nc.sync.dma_start(out=weights_raw, in_=wb[:, :])
            weights = singles.tile([128, WF], f32r)
            nc.scalar.activation(out=weights, in_=weights_raw, func=AF.Copy)
            P = pspool.tile([128, 4096], f32, name="P")
            LGW = G_SUPER * F_OUT
            logits_all = singles.tile([128, n_super * LGW], f32,
                                      name="lgall")

            sig_v = sig.rearrange("(T n) c -> T n c", n=SUPER)
            # transposed columns are sequential samples: sample = T*2048 +
            # 128*g + p lands at logits partition p, group g.
            out_v = out.rearrange("(T g p) o -> T p g o", p=128, g=G_SUPER)

            # batched log-softmax over a half of the supers at a time:
            # per-op bubble overheads amortize across the wide ops.
            w_bp = _w_raw(weights_raw, "bp")               # [128, 12]

            def phase_b(H, hsup):
                grp = hsup * G_SUPER
                ch3 = logits_all[:, H * LGW:(H + hsup) * LGW].rearrange(
                    "p (g o) -> p g o", o=F_OUT)
                bp3d = bass.AP(tensor=w_bp.tensor, offset=w_bp.offset,
                               ap=[w_bp.ap[0], [0, grp], w_bp.ap[1]])
                lb = outp.tile([128, grp, F_OUT], f32, tag="lb")
                nc.vector.tensor_tensor(out=lb, in0=ch3, in1=bp3d, op=ALU.add)
                e = outp.tile([128, grp, F_OUT], f32, tag="e")
                nc.scalar.activation(out=e, in_=lb, func=AF.Exp)
                ssum = outp.tile([128, grp], f32, tag="ss")
                nc.vector.tensor_reduce(out=ssum, in_=e,
                                        axis=mybir.AxisListType.X, op=ALU.add)
                lse = outp.tile([128, grp], f32, tag="lse")
                nc.scalar.activation(out=lse, in_=ssum, func=AF.Ln)
                lse3 = bass.AP(tensor=lse.tensor, offset=lse.offset,
                               ap=[lse.ap[0], lse.ap[1], [0, F_OUT]])
                ot = outp.tile([128, grp, F_OUT], f32, tag="ot")
                nc.vector.tensor_tensor(out=ot, in0=lb, in1=lse3,
                                        op=ALU.subtract)
                for T in range(H, H + hsup):
                    nc.sync.dma_start(
                        out=out_v[T],
                        in_=ot[:, (T - H) * G_SUPER:(T - H + 1) * G_SUPER, :])

            xt = {}

            def emit_transposes(T):
                xA = sbx.tile([128, SUPER], bf16, tag="xA")
                xB = sbx.tile([128, SUPER], bf16, tag="xB")
                x2A = sbx.tile([128, SUPER], bf16, tag="x2A")
                x2B = sbx.tile([64, SUPER], bf16, tag="x2B")
                # half-super granularity so the first sub-tile's matmuls can
                # start ~4us earlier and transposes overlap compute.
                for h in range(2):
                    hs = slice(h * (SUPER // 2), (h + 1) * (SUPER // 2))
                    nc.sync.dma_start_transpose(out=xA[:, hs],
                                                in_=sig_v[T][hs, 0:128])
                    nc.sync.dma_start_transpose(out=xB[:, hs],
                                                in_=sig_v[T][hs, 128:256])
                    nc.gpsimd.tensor_tensor(out=x2A[:, hs], in0=xA[:, hs],
                                            in1=xA[:, hs], op=ALU.mult)
                    nc.gpsimd.tensor_tensor(out=x2B[:, hs],
                                            in0=xB[0:64, hs],
                                            in1=xB[0:64, hs], op=ALU.mult)
                xt[T] = (xA, xB, x2A, x2B)

            def emit_ssum(st):
                # window-sum matmuls for global sub-tile st; emitted one
                # sub-tile early (between conv2 and fc1 of st-1) so the PE
                # stays busy while the h2 evacuation runs on DVE.
                T, q = divmod(st, SUBQ)
                xA, xB, x2A, x2B = xt[T]
                nsl = slice(q * NSUB, (q + 1) * NSUB)
                s_sl = P[0:120, _PS_S[st % 2]:_PS_S[st % 2] + NSUB]
                nc.tensor.matmul(s_sl, _w16(weights16, "sA_a"),
                                 xA[:, nsl], start=True, stop=False)
                nc.tensor.matmul(s_sl, _w16(weights16, "sA_b"),
                                 xB[0:64, nsl], start=False, stop=True)
                s2_sl = P[0:120, _PS_S2[st % 2]:_PS_S2[st % 2] + NSUB]
                nc.tensor.matmul(s2_sl, _w16(weights16, "s2_a"),
                                 x2A[:, nsl], start=True, stop=False)
                nc.tensor.matmul(s2_sl, _w16(weights16, "s2_b"),
                                 x2B[:, nsl], start=False, stop=True)
                return s_sl, s2_sl

            emit_transposes(0)
            n_st = n_super * SUBQ
            pend = emit_ssum(0)
            for st in range(n_st):
                T, q = divmod(st, SUBQ)
                lg_sl = P[:, _PS_LG:_PS_LG + G_SUPER * F_OUT]
                if q == 5 and T + 1 < n_super:
                    emit_transposes(T + 1)
                s_sl, s2_sl = pend

                if True:
                    # ---- std = sqrt((s2 - s'^2/10)/9) ----
                    # s' is the unscaled window sum; Square's input scale
                    # sqrt(0.1) applies the 1/10.  Row 120 of u is memset to
                    # 9.0 (sub overwrites only rows 0..119), so the sqrt
                    # emits an exact 1.0 ones row that carries the bias
                    # folds downstream; std itself is the FP32r-rounding
                    # producer the conv1 matmul needs.
                    t_sb = sbh.tile([120, NSUB], f32, tag="t")
                    nc.scalar.activation(out=t_sb, in_=s_sl, func=AF.Square,
                                         scale=math.sqrt(0.1))
                    u_sb = sbh.tile([121, NSUB], f32, tag="u")
                    nc.gpsimd.memset(u_sb[96:121, :], 9.0)
                    nc.vector.tensor_tensor(out=u_sb[0:120, :], in0=s2_sl,
                                            in1=t_sb, op=ALU.subtract)
                    std = sbh.tile([121, NSUB], f32r, tag="std")
                    nc.scalar.activation(out=std, in_=u_sb,
                                         func=AF.Sqrt, scale=1.0 / 9.0)

                    # ---- conv1 (121 -> 2x128) ----
                    h1_sl = P[:, _PS_H1:_PS_H1 + 2 * NSUB]
                    nc.tensor.matmul(h1_sl[:, 0:NSUB], _w(weights, "c1_a"),
                                     std, start=True, stop=True)
                    nc.tensor.matmul(h1_sl[:, NSUB:2 * NSUB],
                                     _w(weights, "c1_b"), std,
                                     start=True, stop=True)
                    h1 = sbh.tile([128, 2 * NSUB], f32r, tag="h1")
                    nc.scalar.activation(out=h1, in_=h1_sl, func=AF.Relu)

                    # ---- conv2 (190 -> 360 + ones) ----
                    h2_sl = P[:, _PS_H2[st % 2]:_PS_H2[st % 2] + 3 * NSUB]
                    for m in range(3):
                        o_sl = h2_sl[:, m * NSUB:(m + 1) * NSUB]
                        nc.tensor.matmul(o_sl, _w(weights, f"c2a{m}"),
                                         h1[:, 0:NSUB],
                                         start=True, stop=False)
                        nc.tensor.matmul(o_sl, _w(weights, f"c2b{m}"),
                                         h1[0:63, NSUB:2 * NSUB],
                                         start=False, stop=True)
                    h2 = sbh.tile([128, 3 * NSUB], f32r, tag="h2")
                    nc.vector.tensor_scalar(out=h2, in0=h2_sl, scalar1=0.0,
                                            scalar2=None, op0=ALU.max)

                    # next sub-tile's window sums keep the PE busy while the
                    # h2 evacuation runs on DVE
                    if st + 1 < n_st:
                        pend = emit_ssum(st + 1)

                    # ---- fc1 (360 -> 256) ----
                    h3_sl = P[:, _PS_H3:_PS_H3 + 2 * NSUB]
                    kr = ((128, 0), (128, 1), (105, 2))
                    for m in range(2):
                        o_sl = h3_sl[:, m * NSUB:(m + 1) * NSUB]
                        for k, (kc, ki) in enumerate(kr):
                            nc.tensor.matmul(
                                o_sl, _w(weights, f"f1_{ki}_{m}"),
                                h2[0:kc, ki * NSUB:(ki + 1) * NSUB],
                                start=(k == 0), stop=(k == 2))
                    h3 = sbh.tile([128, 2 * NSUB], f32r, tag="h3")
                    nc.vector.tensor_scalar(out=h3, in0=h3_sl, scalar1=0.0,
                                            scalar2=None, op0=ALU.max)

                    # ---- fc2 (256 -> 64) + bias + relu ----
                    h4_sl = P[0:F_FC2, _PS_H4:_PS_H4 + NSUB]
                    for k in range(2):
                        nc.tensor.matmul(h4_sl, _w(weights, f"f2_{k}"),
                                         h3[:, k * NSUB:(k + 1) * NSUB],
                                         start=(k == 0), stop=(k == 1))
                    h4 = sbh.tile([F_FC2, NSUB], f32r, tag="h4")
                    nc.scalar.activation(out=h4, in_=h4_sl, func=AF.Relu,
                                         bias=_w_raw(weights_raw, "b4"))

                    # ---- pamap via weight streaming: logits sample-major ----
                    for j in range(2):
                        nc.tensor.matmul(
                            lg_sl[:, q * 24 + j * F_OUT:
                                  q * 24 + (j + 1) * F_OUT],
                            h4[:, j * 128:(j + 1) * 128],
                            _w(weights, "wp"), start=True, stop=True)

                if q == SUBQ - 1:
                    nc.vector.tensor_copy(
                        out=logits_all[:, T * LGW:(T + 1) * LGW], in_=lg_sl)
                    if T == n_super // 2 - 1 and n_super > 1:
                        # first-half log-softmax interleaves with the main
                        # loop (costs 2 ACT table reloads, hides ~5us tail)
                        phase_b(0, n_super // 2)

            # second-half log-softmax (first half ran mid-loop)
            H0 = n_super // 2 if n_super > 1 else 0
            phase_b(H0, n_super - H0)

    _split_excess_waits(nc)
    return nc


def _get_program(n_super):
    global _PROGRAM
    if _PROGRAM is None or _PROGRAM[0] != n_super:
        _PROGRAM = (n_super, _build_program(n_super))
    return _PROGRAM[1]


# ---------------------------------------------------------------------------
# Entry point
# ---------------------------------------------------------------------------
def kernel(signal, conv1_w, conv1_b, conv2_w, conv2_b, fc1_w, fc1_b,
           fc2_w, fc2_b, pamap_w, pamap_b, **_unused):
    from concourse.bass_utils import run_bass_kernel_spmd

    n_super = _N_SUPER
    b_core = n_super * SUPER
    signal = np.asarray(signal, np.float32)
    b_tot = signal.shape[0]
    assert b_tot == b_core * NCORES, (b_tot, b_core)

    blob, blob16 = _build_blob(np.asarray(conv1_w), np.asarray(conv1_b),
                               np.asarray(conv2_w), np.asarray(conv2_b),
                               np.asarray(fc1_w), np.asarray(fc1_b),
                               np.asarray(fc2_w), np.asarray(fc2_b),
                               np.asarray(pamap_w), np.asarray(pamap_b))

    nc = _get_program(n_super)
    sig_flat = np.ascontiguousarray(signal.reshape(b_tot, F_IN))
    sig_prep = _prep_sig(sig_flat)
    in_maps = [{"sig": sig_prep[c * b_core:(c + 1) * b_core], "wb": blob,
                "wb16": blob16}
               for c in range(NCORES)]
    res = run_bass_kernel_spmd(nc, in_maps, core_ids=list(range(NCORES)))
    outs = [res.results[c]["out"] for c in range(NCORES)]
    return np.concatenate(outs, axis=0)
